# revision 21
# baseline (speedup 1.0000x reference)
"""Trainium2 Bass kernel for nn_CrossAttention (B=4, NQ=1024, NK=2048, dim=512,
ctx_dim=1024, H=8, head_dim=64, scale=dim**-0.5).

Sharding: 8 cores = batch (4) x head-group (2).  Core c handles batch b=c//2
and heads [hg*4, hg*4+4) where hg=c%2.  Each core computes its 4 heads'
projections, scores, softmax and attention-weighted values; outputs are
disjoint so no collectives are needed.

Per-core dataflow (all fp32; matmuls run as float32r at full PE rate):
  phase 0: PE-transpose q[b] and context[b] (via identity matmuls), then
           project to qh^T [256,1024], kh^T [256,2048] (head-dim on
           partitions) and vh [2048, 4x(64+1)] (ones column appended per head
           to get attention row-sums for free in the AV matmul).
  main, per head:
    A-side: S[q,k] tiles (qh^T stationary) -> exp on ACT (row-sum via
            accum_out) -> normalize on DVE -> DMA attn weights out.
    B-side: S^T[k,q] tiles (kh^T stationary) -> exp on ACT -> AV matmul
            (vh stationary) accumulating ctx^T [65, q] in PSUM; row 64 is the
            softmax denominator; divide on DVE; DMA ctx^T out (host
            transposes the small [256,1024] per-core result).
"""

from contextlib import ExitStack

import numpy as np

import concourse.bacc as bacc
import concourse.bass as bass
import concourse.mybir as mybir
import concourse.tile as tile
from concourse.bass_utils import run_bass_kernel_spmd
from concourse.masks import make_identity

F32 = mybir.dt.float32
F32R = mybir.dt.float32r
AF = mybir.ActivationFunctionType

B, NQ, NK = 4, 1024, 2048
DIM, CTX_DIM = 512, 1024
NUM_HEADS = 8
HEAD_DIM = DIM // NUM_HEADS  # 64
SCALE = DIM ** (-0.5)
HPC = 4                      # heads per core
DH = HPC * HEAD_DIM          # 256 projected cols per core
N_CORES = 8

_CACHED_NC = None


def _build_nc():
    nc = bacc.Bacc("TRN2", target_bir_lowering=False, debug=False)

    qx = nc.dram_tensor("qx", [NQ, DIM], F32, kind="ExternalInput").ap()
    cx = nc.dram_tensor("cx", [NK, CTX_DIM], F32, kind="ExternalInput").ap()
    wq = nc.dram_tensor("wq", [DIM, DH], F32, kind="ExternalInput").ap()
    wk = nc.dram_tensor("wk", [CTX_DIM, DH], F32, kind="ExternalInput").ap()
    wv = nc.dram_tensor("wv", [CTX_DIM, DH], F32, kind="ExternalInput").ap()
    attn_o = nc.dram_tensor("attn_o", [HPC, NQ, NK], F32, kind="ExternalOutput").ap()
    # context-vector slice, transposed: [DH, NQ] (host transposes back)
    cvt_o = nc.dram_tensor("cvt_o", [DH, NQ], F32, kind="ExternalOutput").ap()

    with tile.TileContext(nc) as tc:
        with ExitStack() as ctx:
            _emit(ctx, tc, qx, cx, wq, wk, wv, attn_o, cvt_o)
    nc.compile()
    return nc


def _emit(ctx, tc, qx, cx, wq, wk, wv, attn_o, cvt_o):
    nc = tc.nc
    QCC = DIM // 128       # 4 c-chunks for q projection
    CCC = CTX_DIM // 128   # 8 c-chunks for k/v projection
    NQT = NQ // 128        # 8 q row tiles
    NKT = NK // 128        # 16 k row tiles
    NKB = NK // 512        # 4 score col blocks
    NQB = NQ // 512        # 2 q col blocks (for S^T / AV)
    HW = HEAD_DIM + 1      # 65: head block incl. ones column

    wpool = ctx.enter_context(tc.tile_pool(name="w", bufs=1))
    proj = ctx.enter_context(tc.tile_pool(name="proj", bufs=1))
    ld = ctx.enter_context(tc.tile_pool(name="ld", bufs=4))
    big = ctx.enter_context(tc.tile_pool(name="big", bufs=8))
    med = ctx.enter_context(tc.tile_pool(name="med", bufs=4))
    smal = ctx.enter_context(tc.tile_pool(name="smal", bufs=2))
    ps_av = ctx.enter_context(tc.tile_pool(name="ps_av", bufs=2, space="PSUM"))

    # ---- constants / weights -------------------------------------------------
    ident = wpool.tile([128, 128], F32, tag="ident")
    make_identity(nc, ident)

    # Raw f32 weights stage through the transient cld-tagged slots, then an
    # ACT copy rounds them into the persistent f32r tiles.
    wq_t = wpool.tile([128, QCC, DH], F32R, tag="wq")
    wk_t = wpool.tile([128, CCC, DH], F32R, tag="wk")
    wv_t = wpool.tile([128, CCC, DH], F32R, tag="wv")
    wqr = ld.tile([128, QCC, DH], F32, tag="cld", name="wqr")
    nc.sync.dma_start(out=wqr, in_=wq.rearrange("(cc p) d -> p cc d", p=128))
    nc.scalar.copy(wq_t, wqr)
    for half in range(2):
        wkr = ld.tile([128, CCC // 2, DH], F32, tag="cld", name=f"wkr{half}")
        nc.sync.dma_start(
            out=wkr,
            in_=wk.rearrange("(cc p) d -> p cc d", p=128)[:, half * 4:half * 4 + 4, :])
        nc.scalar.copy(wk_t[:, half * 4:half * 4 + 4, :], wkr)
        wvr = ld.tile([128, CCC // 2, DH], F32, tag="cld", name=f"wvr{half}")
        nc.sync.dma_start(
            out=wvr,
            in_=wv.rearrange("(cc p) d -> p cc d", p=128)[:, half * 4:half * 4 + 4, :])
        nc.scalar.copy(wv_t[:, half * 4:half * 4 + 4, :], wvr)

    # phase-0 PSUM: its own 4-bank pool (closed before the main loop opens
    # the 4-bank S pool), so transposes/projection chunks pipeline through
    # per-iteration tiles instead of a manually rotated region.
    ps_tr_cm = tc.tile_pool(name="ps_tr", bufs=4, space="PSUM")
    ps_tr = ps_tr_cm.__enter__()

    def transpose_into(dst, src, rot):
        """PE-transpose one [128,128] block via a pool PSUM tile; drain
        copies alternate between DVE and ACT to double throughput."""
        bank = ps_tr.tile([128, 128], F32, tag="tr", name=f"tr{rot}")
        nc.tensor.transpose(bank, src, ident)
        nc.vector.tensor_copy(dst, bank)

    # ---- phase 0a: qT and qh^T ----------------------------------------------
    rot = 0
    qT = [med.tile([128, NQ], F32R, tag="med", name=f"qT{j}") for j in range(QCC)]
    for i in range(NQT):
        qld = ld.tile([128, DIM], F32, tag="qld")
        nc.sync.dma_start(out=qld, in_=qx[i * 128:(i + 1) * 128, :])
        for j in range(QCC):
            transpose_into(qT[j][:, i * 128:(i + 1) * 128],
                           qld[:, j * 128:(j + 1) * 128], rot)
            rot += 1

    qhT = proj.tile([128, 2, NQ], F32R, tag="qhT")  # [d'(2x128), q]
    for m in range(2):
        for qb in range(NQB):
            ps = ps_tr.tile([128, 512], F32, tag="tr", name=f"qp{m}_{qb}")
            for cc in range(QCC):
                nc.tensor.matmul(
                    ps,
                    lhsT=wq_t[:, cc, m * 128:(m + 1) * 128],
                    rhs=qT[cc][:, qb * 512:(qb + 1) * 512],
                    start=(cc == 0), stop=(cc == QCC - 1))
            nc.scalar.copy(qhT[:, m, qb * 512:(qb + 1) * 512], ps)

    # ---- phase 0b: ctxT + vh interleaved per k row-tile ---------------------
    ctxT = [big.tile([128, NK], F32R, tag="big", name=f"ctxT{j}") for j in range(CCC)]
    khT = proj.tile([128, 2, NK], F32R, tag="khT")  # [d'(2x128), k]
    vha = proj.tile([128, NKT, HPC * HW], F32R, tag="vha")  # [k%128, kt, h*(64+1)]
    ones_f = smal.tile([128, NKT * HPC], F32, tag="ones", bufs=1)
    nc.vector.memset(ones_f, 1.0)
    ones_cols = vha.rearrange("p t (h e) -> p t h e", e=HW)[:, :, :, HEAD_DIM]
    nc.scalar.copy(ones_cols, ones_f.rearrange("p (t h) -> p t h", t=NKT))
    for i in range(NKT):
        cld = ld.tile([128, CTX_DIM], F32, tag="cld")
        nc.sync.dma_start(out=cld, in_=cx[i * 128:(i + 1) * 128, :])
        for j in range(CCC):
            transpose_into(ctxT[j][:, i * 128:(i + 1) * 128],
                           cld[:, j * 128:(j + 1) * 128], rot)
            rot += 1
        # vh row-tile i only needs ctxT[:, i-slice], which is now in flight
        ps = ps_av.tile([128, DH], F32, tag="av", name=f"vps{i}")
        for cc in range(CCC):
            nc.tensor.matmul(
                ps,
                lhsT=ctxT[cc][:, i * 128:(i + 1) * 128],
                rhs=wv_t[:, cc, :],
                start=(cc == 0), stop=(cc == CCC - 1))
        dst = vha.rearrange("p t (h e) -> p t h e", e=HW)[:, i, :, :HEAD_DIM]
        nc.scalar.copy(dst, ps)
        if i % 4 == 3:
            # kh^T for this 512-wide k block only needs the 4 row-tiles just
            # transposed, so the B-side exp pipeline can start early.
            kb = i // 4
            for m in range(2):
                ps2 = ps_tr.tile([128, 512], F32, tag="tr", name=f"kp{kb}_{m}")
                for cc in range(CCC):
                    nc.tensor.matmul(
                        ps2,
                        lhsT=wk_t[:, cc, m * 128:(m + 1) * 128],
                        rhs=ctxT[cc][:, kb * 512:(kb + 1) * 512],
                        start=(cc == 0), stop=(cc == CCC - 1))
                nc.scalar.copy(khT[:, m, kb * 512:(kb + 1) * 512], ps2)

    ps_tr_cm.__exit__(None, None, None)
    ps_st = ctx.enter_context(tc.tile_pool(name="ps_st", bufs=2, space="PSUM"))
    ps_s = ctx.enter_context(tc.tile_pool(name="ps_s", bufs=1, space="PSUM"))

    # ---- main loop: heads software-pipelined with lag 1 ---------------------
    # B(h) produces softmax denominators (ones column); A(h) consumes them.
    # A(h-1) score tiles are interleaved between B(h) S^T tiles at 1:2 ratio
    # so the in-order ACT queue always has ready exp work.
    rrs_by_head = {}
    av_by_head = {}

    def head_mo(h):
        return (h * HEAD_DIM) % 128, (h * HEAD_DIM) // 128

    def emit_b_kt(h, kt):
        off, m = head_mo(h)
        av_ps = av_by_head[h]
        st_ps = ps_st.tile([128, NQ], F32, tag="st", name=f"st{h}_{kt}")
        for qb in range(NQB):
            nc.tensor.matmul(
                st_ps[:, qb * 512:(qb + 1) * 512],
                lhsT=khT[off:off + HEAD_DIM, m, kt * 128:(kt + 1) * 128],
                rhs=qhT[off:off + HEAD_DIM, m, qb * 512:(qb + 1) * 512],
                start=True, stop=True)
        est = med.tile([128, NQ], F32R, tag="med", name=f"est{h}_{kt}")
        nc.scalar.activation(out=est, in_=st_ps, func=AF.Exp, scale=SCALE)
        for qb in range(NQB):
            nc.tensor.matmul(
                av_ps[qb],
                lhsT=vha[:, kt, h * HW:(h + 1) * HW],
                rhs=est[:, qb * 512:(qb + 1) * 512],
                start=(kt == 0), stop=(kt == NKT - 1))

    def emit_b_tail(h):
        av_ps = av_by_head.pop(h)
        rrs = []
        for qb in range(NQB):
            cv = smal.tile([HW, 512], F32, tag="cv", name=f"cv{h}_{qb}")
            nc.vector.tensor_copy(cv, av_ps[qb])
            rr = smal.tile([1, 512], F32, tag="rr", name=f"rr{h}_{qb}", bufs=4)
            rrs.append(rr)
            nc.vector.reciprocal(rr, cv[HEAD_DIM:HW, :])
            cvf = smal.tile([HEAD_DIM, 512], F32, tag="cvf", name=f"cvf{h}_{qb}")
            rrb = smal.tile([HEAD_DIM, 512], F32, tag="rrb", name=f"rrb{h}_{qb}")
            nc.gpsimd.partition_broadcast(rrb, rr)
            nc.vector.tensor_tensor(out=cvf, in0=cv[:HEAD_DIM, :], in1=rrb,
                                    op=mybir.AluOpType.mult)
            nc.sync.dma_start(
                out=cvt_o[h * HEAD_DIM:(h + 1) * HEAD_DIM,
                          qb * 512:(qb + 1) * 512],
                in_=cvf)
        rrs_by_head[h] = rrs

    def emit_a_qt(h, qt):
        off, m = head_mo(h)
        rrs = rrs_by_head[h]
        rcp = smal.tile([128, 1], F32, tag="rcp", name=f"rcp{h}_{qt}", bufs=4)
        nc.sync.dma_start(
            out=rcp,
            in_=rrs[qt // 4][0:1, (qt % 4) * 128:(qt % 4) * 128 + 128])
        exps = big.tile([128, NK], F32, tag="big", name=f"exps{h}_{qt}")
        for half in range(2):
            s_ps = ps_s.tile([128, NK // 2], F32, tag="s",
                             name=f"s{h}_{qt}_{half}")
            for kb2 in range(2):
                kb = half * 2 + kb2
                nc.tensor.matmul(
                    s_ps[:, kb2 * 512:(kb2 + 1) * 512],
                    lhsT=qhT[off:off + HEAD_DIM, m, qt * 128:(qt + 1) * 128],
                    rhs=khT[off:off + HEAD_DIM, m, kb * 512:(kb + 1) * 512],
                    start=True, stop=True)
            nc.scalar.activation(
                out=exps[:, half * (NK // 2):(half + 1) * (NK // 2)],
                in_=s_ps, func=AF.Exp, scale=SCALE)
        attn = big.tile([128, NK], F32, tag="big", name=f"attn{h}_{qt}")
        nc.vector.tensor_scalar_mul(attn, exps, rcp)
        nc.sync.dma_start(out=attn_o[h, qt * 128:(qt + 1) * 128, :], in_=attn)

    for h in range(HPC):
        av_by_head[h] = [ps_av.tile([HW, 512], F32, tag="av", name=f"av{h}_{qb}")
                         for qb in range(NQB)]
        for kt in range(NKT):
            emit_b_kt(h, kt)
            if h >= 1 and kt % 2 == 1:
                emit_a_qt(h - 1, kt // 2)
        emit_b_tail(h)
    for qt in range(NQT):
        emit_a_qt(HPC - 1, qt)


def kernel(q, context, Wq, Wk, Wv):
    global _CACHED_NC
    q = np.asarray(q, dtype=np.float32)
    context = np.asarray(context, dtype=np.float32)
    Wq = np.asarray(Wq, dtype=np.float32)
    Wk = np.asarray(Wk, dtype=np.float32)
    Wv = np.asarray(Wv, dtype=np.float32)

    if _CACHED_NC is None:
        _CACHED_NC = _build_nc()
    nc = _CACHED_NC

    in_maps = []
    for c in range(N_CORES):
        b, hg = c // 2, c % 2
        cols = slice(hg * DH, (hg + 1) * DH)
        in_maps.append({
            "qx": np.ascontiguousarray(q[b]),
            "cx": np.ascontiguousarray(context[b]),
            "wq": np.ascontiguousarray(Wq[:, cols]),
            "wk": np.ascontiguousarray(Wk[:, cols]),
            "wv": np.ascontiguousarray(Wv[:, cols]),
        })

    res = run_bass_kernel_spmd(nc, in_maps, list(range(N_CORES)))

    attn = np.empty((B, NUM_HEADS, NQ, NK), np.float32)
    ctxv = np.empty((B, NQ, DIM), np.float32)
    for c in range(N_CORES):
        b, hg = c // 2, c % 2
        attn[b, hg * HPC:(hg + 1) * HPC] = res.results[c]["attn_o"]
        ctxv[b, :, hg * DH:(hg + 1) * DH] = res.results[c]["cvt_o"].T
    return ctxv, attn


# revision 23
# speedup vs baseline: 1.0139x; 1.0139x over previous
"""Trainium2 Bass kernel for nn_CrossAttention (B=4, NQ=1024, NK=2048, dim=512,
ctx_dim=1024, H=8, head_dim=64, scale=dim**-0.5).

Sharding: 8 cores = batch (4) x head-group (2).  Core c handles batch b=c//2
and heads [hg*4, hg*4+4) where hg=c%2.  Each core computes its 4 heads'
projections, scores, softmax and attention-weighted values; outputs are
disjoint so no collectives are needed.

Per-core dataflow (all fp32; matmuls run as float32r at full PE rate):
  phase 0: PE-transpose q[b] and context[b] (via identity matmuls), then
           project to qh^T [256,1024], kh^T [256,2048] (head-dim on
           partitions) and vh [2048, 4x(64+1)] (ones column appended per head
           to get attention row-sums for free in the AV matmul).
  main, per head:
    A-side: S[q,k] tiles (qh^T stationary) -> exp on ACT (row-sum via
            accum_out) -> normalize on DVE -> DMA attn weights out.
    B-side: S^T[k,q] tiles (kh^T stationary) -> exp on ACT -> AV matmul
            (vh stationary) accumulating ctx^T [65, q] in PSUM; row 64 is the
            softmax denominator; divide on DVE; DMA ctx^T out (host
            transposes the small [256,1024] per-core result).
"""

from contextlib import ExitStack

import numpy as np

import concourse.bacc as bacc
import concourse.bass as bass
import concourse.mybir as mybir
import concourse.tile as tile
from concourse.bass_utils import run_bass_kernel_spmd
from concourse.masks import make_identity

F32 = mybir.dt.float32
F32R = mybir.dt.float32r
AF = mybir.ActivationFunctionType

B, NQ, NK = 4, 1024, 2048
DIM, CTX_DIM = 512, 1024
NUM_HEADS = 8
HEAD_DIM = DIM // NUM_HEADS  # 64
SCALE = DIM ** (-0.5)
HPC = 4                      # heads per core
DH = HPC * HEAD_DIM          # 256 projected cols per core
N_CORES = 8

_CACHED_NC = None


def _build_nc():
    nc = bacc.Bacc("TRN2", target_bir_lowering=False, debug=False)

    qx = nc.dram_tensor("qx", [NQ, DIM], F32, kind="ExternalInput").ap()
    cx = nc.dram_tensor("cx", [NK, CTX_DIM], F32, kind="ExternalInput").ap()
    wq = nc.dram_tensor("wq", [DIM, DH], F32, kind="ExternalInput").ap()
    wk = nc.dram_tensor("wk", [CTX_DIM, DH], F32, kind="ExternalInput").ap()
    wv = nc.dram_tensor("wv", [CTX_DIM, DH], F32, kind="ExternalInput").ap()
    attn_o = nc.dram_tensor("attn_o", [HPC, NQ, NK], F32, kind="ExternalOutput").ap()
    # context-vector slice, transposed: [DH, NQ] (host transposes back)
    cvt_o = nc.dram_tensor("cvt_o", [DH, NQ], F32, kind="ExternalOutput").ap()

    with tile.TileContext(nc) as tc:
        with ExitStack() as ctx:
            _emit(ctx, tc, qx, cx, wq, wk, wv, attn_o, cvt_o)
    nc.compile()
    return nc


def _emit(ctx, tc, qx, cx, wq, wk, wv, attn_o, cvt_o):
    nc = tc.nc
    QCC = DIM // 128       # 4 c-chunks for q projection
    CCC = CTX_DIM // 128   # 8 c-chunks for k/v projection
    NQT = NQ // 128        # 8 q row tiles
    NKT = NK // 128        # 16 k row tiles
    NKB = NK // 512        # 4 score col blocks
    NQB = NQ // 512        # 2 q col blocks (for S^T / AV)
    HW = HEAD_DIM + 1      # 65: head block incl. ones column

    wpool = ctx.enter_context(tc.tile_pool(name="w", bufs=1))
    proj = ctx.enter_context(tc.tile_pool(name="proj", bufs=1))
    ld = ctx.enter_context(tc.tile_pool(name="ld", bufs=4))
    big = ctx.enter_context(tc.tile_pool(name="big", bufs=8))
    med = ctx.enter_context(tc.tile_pool(name="med", bufs=4))
    smal = ctx.enter_context(tc.tile_pool(name="smal", bufs=2))
    ps_av = ctx.enter_context(tc.tile_pool(name="ps_av", bufs=2, space="PSUM"))

    # ---- constants / weights -------------------------------------------------
    ident_f = wpool.tile([128, 128], F32, tag="ident_f")
    make_identity(nc, ident_f)
    ident = wpool.tile([128, 128], F32R, tag="ident")
    nc.vector.tensor_copy(ident, ident_f)

    # Raw f32 weights stage through the transient cld-tagged slots, then an
    # ACT copy rounds them into the persistent f32r tiles.
    wq_t = wpool.tile([128, QCC, DH], F32R, tag="wq")
    wk_t = wpool.tile([128, CCC, DH], F32R, tag="wk")
    wv_t = wpool.tile([128, CCC, DH], F32R, tag="wv")
    wqr = ld.tile([128, QCC, DH], F32, tag="cld", name="wqr")
    nc.sync.dma_start(out=wqr, in_=wq.rearrange("(cc p) d -> p cc d", p=128))
    nc.scalar.copy(wq_t, wqr)
    for half in range(2):
        wkr = ld.tile([128, CCC // 2, DH], F32, tag="cld", name=f"wkr{half}")
        nc.sync.dma_start(
            out=wkr,
            in_=wk.rearrange("(cc p) d -> p cc d", p=128)[:, half * 4:half * 4 + 4, :])
        nc.scalar.copy(wk_t[:, half * 4:half * 4 + 4, :], wkr)
        wvr = ld.tile([128, CCC // 2, DH], F32, tag="cld", name=f"wvr{half}")
        nc.sync.dma_start(
            out=wvr,
            in_=wv.rearrange("(cc p) d -> p cc d", p=128)[:, half * 4:half * 4 + 4, :])
        nc.scalar.copy(wv_t[:, half * 4:half * 4 + 4, :], wvr)

    # phase-0 PSUM: its own 4-bank pool (closed before the main loop opens
    # the 4-bank S pool), so transposes/projection chunks pipeline through
    # per-iteration tiles instead of a manually rotated region.
    ps_tr_cm = tc.tile_pool(name="ps_tr", bufs=4, space="PSUM")
    ps_tr = ps_tr_cm.__enter__()

    def transpose_into(dst, src, rot):
        """PE-transpose one [128,128] block via a pool PSUM tile; drain
        copies alternate between DVE and ACT to double throughput."""
        bank = ps_tr.tile([128, 128], F32R, tag="tr", name=f"tr{rot}")
        nc.tensor.transpose(bank, src, ident)
        nc.vector.tensor_copy(dst, bank)

    # ---- phase 0a: qT and qh^T ----------------------------------------------
    rot = 0
    qT = [med.tile([128, NQ], F32R, tag="med", name=f"qT{j}") for j in range(QCC)]
    for i in range(NQT):
        qld = ld.tile([128, DIM], F32R, tag="qld")
        nc.sync.dma_start(out=qld, in_=qx[i * 128:(i + 1) * 128, :].bitcast(F32R))
        for j in range(QCC):
            transpose_into(qT[j][:, i * 128:(i + 1) * 128],
                           qld[:, j * 128:(j + 1) * 128], rot)
            rot += 1

    qhT = proj.tile([128, 2, NQ], F32R, tag="qhT")  # [d'(2x128), q]
    for m in range(2):
        for qb in range(NQB):
            ps = ps_tr.tile([128, 512], F32, tag="tr", name=f"qp{m}_{qb}")
            for cc in range(QCC):
                nc.tensor.matmul(
                    ps,
                    lhsT=wq_t[:, cc, m * 128:(m + 1) * 128],
                    rhs=qT[cc][:, qb * 512:(qb + 1) * 512],
                    start=(cc == 0), stop=(cc == QCC - 1))
            nc.scalar.copy(qhT[:, m, qb * 512:(qb + 1) * 512], ps)

    # ---- phase 0b: ctxT + vh interleaved per k row-tile ---------------------
    ctxT = [big.tile([128, NK], F32R, tag="big", name=f"ctxT{j}") for j in range(CCC)]
    khT = proj.tile([128, 2, NK], F32R, tag="khT")  # [d'(2x128), k]
    vha = proj.tile([128, NKT, HPC * HW], F32R, tag="vha")  # [k%128, kt, h*(64+1)]
    ones_f = smal.tile([128, NKT * HPC], F32, tag="ones", bufs=1)
    nc.vector.memset(ones_f, 1.0)
    ones_cols = vha.rearrange("p t (h e) -> p t h e", e=HW)[:, :, :, HEAD_DIM]
    nc.scalar.copy(ones_cols, ones_f.rearrange("p (t h) -> p t h", t=NKT))
    for i in range(NKT):
        cld = ld.tile([128, CTX_DIM], F32R, tag="cld")
        nc.sync.dma_start(out=cld, in_=cx[i * 128:(i + 1) * 128, :].bitcast(F32R))
        for j in range(CCC):
            transpose_into(ctxT[j][:, i * 128:(i + 1) * 128],
                           cld[:, j * 128:(j + 1) * 128], rot)
            rot += 1
        # vh row-tile i only needs ctxT[:, i-slice], which is now in flight
        ps = ps_av.tile([128, DH], F32, tag="av", name=f"vps{i}")
        for cc in range(CCC):
            nc.tensor.matmul(
                ps,
                lhsT=ctxT[cc][:, i * 128:(i + 1) * 128],
                rhs=wv_t[:, cc, :],
                start=(cc == 0), stop=(cc == CCC - 1))
        dst = vha.rearrange("p t (h e) -> p t h e", e=HW)[:, i, :, :HEAD_DIM]
        nc.scalar.copy(dst, ps)
        if i % 4 == 3:
            # kh^T for this 512-wide k block only needs the 4 row-tiles just
            # transposed, so the B-side exp pipeline can start early.
            kb = i // 4
            for m in range(2):
                ps2 = ps_tr.tile([128, 512], F32, tag="tr", name=f"kp{kb}_{m}")
                for cc in range(CCC):
                    nc.tensor.matmul(
                        ps2,
                        lhsT=wk_t[:, cc, m * 128:(m + 1) * 128],
                        rhs=ctxT[cc][:, kb * 512:(kb + 1) * 512],
                        start=(cc == 0), stop=(cc == CCC - 1))
                nc.scalar.copy(khT[:, m, kb * 512:(kb + 1) * 512], ps2)

    ps_tr_cm.__exit__(None, None, None)
    ps_st = ctx.enter_context(tc.tile_pool(name="ps_st", bufs=2, space="PSUM"))
    ps_s = ctx.enter_context(tc.tile_pool(name="ps_s", bufs=1, space="PSUM"))

    # ---- main loop: heads software-pipelined with lag 1 ---------------------
    # B(h) produces softmax denominators (ones column); A(h) consumes them.
    # A(h-1) score tiles are interleaved between B(h) S^T tiles at 1:2 ratio
    # so the in-order ACT queue always has ready exp work.
    rrs_by_head = {}
    av_by_head = {}

    def head_mo(h):
        return (h * HEAD_DIM) % 128, (h * HEAD_DIM) // 128

    def emit_b_kt(h, kt):
        off, m = head_mo(h)
        av_ps = av_by_head[h]
        st_ps = ps_st.tile([128, NQ], F32, tag="st", name=f"st{h}_{kt}")
        for qb in range(NQB):
            nc.tensor.matmul(
                st_ps[:, qb * 512:(qb + 1) * 512],
                lhsT=khT[off:off + HEAD_DIM, m, kt * 128:(kt + 1) * 128],
                rhs=qhT[off:off + HEAD_DIM, m, qb * 512:(qb + 1) * 512],
                start=True, stop=True)
        est = med.tile([128, NQ], F32R, tag="med", name=f"est{h}_{kt}")
        nc.scalar.activation(out=est, in_=st_ps, func=AF.Exp, scale=SCALE)
        for qb in range(NQB):
            nc.tensor.matmul(
                av_ps[qb],
                lhsT=vha[:, kt, h * HW:(h + 1) * HW],
                rhs=est[:, qb * 512:(qb + 1) * 512],
                start=(kt == 0), stop=(kt == NKT - 1))

    def emit_b_tail(h):
        av_ps = av_by_head.pop(h)
        rrs = []
        for qb in range(NQB):
            cv = smal.tile([HW, 512], F32, tag="cv", name=f"cv{h}_{qb}")
            nc.vector.tensor_copy(cv, av_ps[qb])
            rr = smal.tile([1, 512], F32, tag="rr", name=f"rr{h}_{qb}", bufs=4)
            rrs.append(rr)
            nc.vector.reciprocal(rr, cv[HEAD_DIM:HW, :])
            cvf = smal.tile([HEAD_DIM, 512], F32, tag="cvf", name=f"cvf{h}_{qb}")
            rrb = smal.tile([HEAD_DIM, 512], F32, tag="rrb", name=f"rrb{h}_{qb}")
            nc.gpsimd.partition_broadcast(rrb, rr)
            nc.vector.tensor_tensor(out=cvf, in0=cv[:HEAD_DIM, :], in1=rrb,
                                    op=mybir.AluOpType.mult)
            nc.sync.dma_start(
                out=cvt_o[h * HEAD_DIM:(h + 1) * HEAD_DIM,
                          qb * 512:(qb + 1) * 512],
                in_=cvf)
        rrs_by_head[h] = rrs

    def emit_a_qt(h, qt):
        off, m = head_mo(h)
        rrs = rrs_by_head[h]
        rcp = smal.tile([128, 1], F32, tag="rcp", name=f"rcp{h}_{qt}", bufs=4)
        nc.sync.dma_start(
            out=rcp,
            in_=rrs[qt // 4][0:1, (qt % 4) * 128:(qt % 4) * 128 + 128])
        exps = big.tile([128, NK], F32, tag="big", name=f"exps{h}_{qt}")
        for half in range(2):
            s_ps = ps_s.tile([128, NK // 2], F32, tag="s",
                             name=f"s{h}_{qt}_{half}")
            for kb2 in range(2):
                kb = half * 2 + kb2
                nc.tensor.matmul(
                    s_ps[:, kb2 * 512:(kb2 + 1) * 512],
                    lhsT=qhT[off:off + HEAD_DIM, m, qt * 128:(qt + 1) * 128],
                    rhs=khT[off:off + HEAD_DIM, m, kb * 512:(kb + 1) * 512],
                    start=True, stop=True)
            nc.scalar.activation(
                out=exps[:, half * (NK // 2):(half + 1) * (NK // 2)],
                in_=s_ps, func=AF.Exp, scale=SCALE)
        attn = big.tile([128, NK], F32, tag="big", name=f"attn{h}_{qt}")
        nc.vector.tensor_scalar_mul(attn, exps, rcp)
        nc.sync.dma_start(out=attn_o[h, qt * 128:(qt + 1) * 128, :], in_=attn)

    for h in range(HPC):
        av_by_head[h] = [ps_av.tile([HW, 512], F32, tag="av", name=f"av{h}_{qb}")
                         for qb in range(NQB)]
        for kt in range(NKT):
            emit_b_kt(h, kt)
            if h >= 1 and kt % 2 == 1:
                emit_a_qt(h - 1, kt // 2)
        emit_b_tail(h)
    for qt in range(NQT):
        emit_a_qt(HPC - 1, qt)


def kernel(q, context, Wq, Wk, Wv):
    global _CACHED_NC
    q = np.asarray(q, dtype=np.float32)
    context = np.asarray(context, dtype=np.float32)
    Wq = np.asarray(Wq, dtype=np.float32)
    Wk = np.asarray(Wk, dtype=np.float32)
    Wv = np.asarray(Wv, dtype=np.float32)

    if _CACHED_NC is None:
        _CACHED_NC = _build_nc()
    nc = _CACHED_NC

    in_maps = []
    for c in range(N_CORES):
        b, hg = c // 2, c % 2
        cols = slice(hg * DH, (hg + 1) * DH)
        in_maps.append({
            "qx": np.ascontiguousarray(q[b]),
            "cx": np.ascontiguousarray(context[b]),
            "wq": np.ascontiguousarray(Wq[:, cols]),
            "wk": np.ascontiguousarray(Wk[:, cols]),
            "wv": np.ascontiguousarray(Wv[:, cols]),
        })

    res = run_bass_kernel_spmd(nc, in_maps, list(range(N_CORES)))

    attn = np.empty((B, NUM_HEADS, NQ, NK), np.float32)
    ctxv = np.empty((B, NQ, DIM), np.float32)
    for c in range(N_CORES):
        b, hg = c // 2, c % 2
        attn[b, hg * HPC:(hg + 1) * HPC] = res.results[c]["attn_o"]
        ctxv[b, :, hg * DH:(hg + 1) * DH] = res.results[c]["cvt_o"].T
    return ctxv, attn


# revision 26
# speedup vs baseline: 38696.1063x; 38165.5650x over previous
"""Trainium2 Bass kernel for nn_CrossAttention (B=4, NQ=1024, NK=2048, dim=512,
ctx_dim=1024, H=8, head_dim=64, scale=dim**-0.5).

Sharding: 8 cores = batch (4) x head-group (2).  Core c handles batch b=c//2
and heads [hg*4, hg*4+4) where hg=c%2.  Each core computes its 4 heads'
projections, scores, softmax and attention-weighted values; outputs are
disjoint so no collectives are needed.

Per-core dataflow (all fp32; matmuls run as float32r at full PE rate):
  phase 0: PE-transpose q[b] and context[b] (via identity matmuls), then
           project to qh^T [256,1024], kh^T [256,2048] (head-dim on
           partitions) and vh [2048, 4x(64+1)] (ones column appended per head
           to get attention row-sums for free in the AV matmul).
  main, per head:
    A-side: S[q,k] tiles (qh^T stationary) -> exp on ACT (row-sum via
            accum_out) -> normalize on DVE -> DMA attn weights out.
    B-side: S^T[k,q] tiles (kh^T stationary) -> exp on ACT -> AV matmul
            (vh stationary) accumulating ctx^T [65, q] in PSUM; row 64 is the
            softmax denominator; divide on DVE; DMA ctx^T out (host
            transposes the small [256,1024] per-core result).
"""

from contextlib import ExitStack

import numpy as np

import concourse.bacc as bacc
import concourse.bass as bass
import concourse.mybir as mybir
import concourse.tile as tile
from concourse.bass_utils import run_bass_kernel_spmd
from concourse.masks import make_identity

F32 = mybir.dt.float32
F32R = mybir.dt.float32r
AF = mybir.ActivationFunctionType

B, NQ, NK = 4, 1024, 2048
DIM, CTX_DIM = 512, 1024
NUM_HEADS = 8
HEAD_DIM = DIM // NUM_HEADS  # 64
SCALE = DIM ** (-0.5)
HPC = 4                      # heads per core
DH = HPC * HEAD_DIM          # 256 projected cols per core
N_CORES = 8

_CACHED_NC = None


def _build_nc():
    nc = bacc.Bacc("TRN2", target_bir_lowering=False, debug=False)

    qx = nc.dram_tensor("qx", [NQ, DIM], F32, kind="ExternalInput").ap()
    cx = nc.dram_tensor("cx", [NK, CTX_DIM], F32, kind="ExternalInput").ap()
    wq = nc.dram_tensor("wq", [DIM, DH], F32, kind="ExternalInput").ap()
    wk = nc.dram_tensor("wk", [CTX_DIM, DH], F32, kind="ExternalInput").ap()
    wv = nc.dram_tensor("wv", [CTX_DIM, DH], F32, kind="ExternalInput").ap()
    attn_o = nc.dram_tensor("attn_o", [HPC, NQ, NK], F32, kind="ExternalOutput").ap()
    # context-vector slice, transposed: [DH, NQ] (host transposes back)
    cvt_o = nc.dram_tensor("cvt_o", [DH, NQ], F32, kind="ExternalOutput").ap()

    with tile.TileContext(nc) as tc:
        with ExitStack() as ctx:
            _emit(ctx, tc, qx, cx, wq, wk, wv, attn_o, cvt_o)
    nc.compile()
    return nc


def _emit(ctx, tc, qx, cx, wq, wk, wv, attn_o, cvt_o):
    nc = tc.nc
    QCC = DIM // 128       # 4 c-chunks for q projection
    CCC = CTX_DIM // 128   # 8 c-chunks for k/v projection
    NQT = NQ // 128        # 8 q row tiles
    NKT = NK // 128        # 16 k row tiles
    NKB = NK // 512        # 4 score col blocks
    NQB = NQ // 512        # 2 q col blocks (for S^T / AV)
    HW = HEAD_DIM + 1      # 65: head block incl. ones column

    wpool = ctx.enter_context(tc.tile_pool(name="w", bufs=1))
    proj = ctx.enter_context(tc.tile_pool(name="proj", bufs=1))
    ld = ctx.enter_context(tc.tile_pool(name="ld", bufs=4))
    big = ctx.enter_context(tc.tile_pool(name="big", bufs=8))
    med = ctx.enter_context(tc.tile_pool(name="med", bufs=4))
    smal = ctx.enter_context(tc.tile_pool(name="smal", bufs=2))
    ps_av = ctx.enter_context(tc.tile_pool(name="ps_av", bufs=2, space="PSUM"))

    # ---- constants / weights -------------------------------------------------
    ident_f = wpool.tile([128, 128], F32, tag="ident_f")
    make_identity(nc, ident_f)
    ident = wpool.tile([128, 128], F32R, tag="ident")
    nc.vector.tensor_copy(ident, ident_f)

    # Raw f32 weights stage through the transient cld-tagged slots, then an
    # ACT copy rounds them into the persistent f32r tiles.
    wq_t = wpool.tile([128, QCC, DH], F32R, tag="wq")
    wk_t = wpool.tile([128, CCC, DH], F32R, tag="wk")
    wv_t = wpool.tile([128, CCC, DH], F32R, tag="wv")
    wqr = ld.tile([128, QCC, DH], F32, tag="cld", name="wqr")
    nc.sync.dma_start(out=wqr, in_=wq.rearrange("(cc p) d -> p cc d", p=128))
    nc.scalar.copy(wq_t, wqr)
    for half in range(2):
        wkr = ld.tile([128, CCC // 2, DH], F32, tag="cld", name=f"wkr{half}")
        nc.sync.dma_start(
            out=wkr,
            in_=wk.rearrange("(cc p) d -> p cc d", p=128)[:, half * 4:half * 4 + 4, :])
        nc.scalar.copy(wk_t[:, half * 4:half * 4 + 4, :], wkr)
        wvr = ld.tile([128, CCC // 2, DH], F32, tag="cld", name=f"wvr{half}")
        nc.sync.dma_start(
            out=wvr,
            in_=wv.rearrange("(cc p) d -> p cc d", p=128)[:, half * 4:half * 4 + 4, :])
        nc.scalar.copy(wv_t[:, half * 4:half * 4 + 4, :], wvr)

    # phase-0 PSUM: its own 4-bank pool (closed before the main loop opens
    # the 4-bank S pool), so transposes/projection chunks pipeline through
    # per-iteration tiles instead of a manually rotated region.
    ps_tr_cm = tc.tile_pool(name="ps_tr", bufs=6, space="PSUM")
    ps_tr = ps_tr_cm.__enter__()

    def transpose_into(dst, src, rot):
        """PE-transpose one [128,128] block via a pool PSUM tile; drain
        copies alternate between DVE and ACT to double throughput."""
        bank = ps_tr.tile([128, 128], F32R, tag="tr", name=f"tr{rot}")
        nc.tensor.transpose(bank, src, ident)
        if rot % 4 == 3:
            nc.scalar.copy(dst, bank)
        else:
            nc.vector.tensor_copy(dst, bank)

    # ---- phase 0a: qT and qh^T ----------------------------------------------
    rot = 0
    qT = [med.tile([128, NQ], F32R, tag="med", name=f"qT{j}") for j in range(QCC)]
    for i in range(NQT):
        qld = ld.tile([128, DIM], F32R, tag="qld")
        nc.sync.dma_start(out=qld, in_=qx[i * 128:(i + 1) * 128, :].bitcast(F32R))
        for j in range(QCC):
            transpose_into(qT[j][:, i * 128:(i + 1) * 128],
                           qld[:, j * 128:(j + 1) * 128], rot)
            rot += 1

    qhT = proj.tile([128, 2, NQ], F32R, tag="qhT")  # [d'(2x128), q]
    for m in range(2):
        for qb in range(NQB):
            ps = ps_tr.tile([128, 512], F32, tag="tr", name=f"qp{m}_{qb}")
            for cc in range(QCC):
                nc.tensor.matmul(
                    ps,
                    lhsT=wq_t[:, cc, m * 128:(m + 1) * 128],
                    rhs=qT[cc][:, qb * 512:(qb + 1) * 512],
                    start=(cc == 0), stop=(cc == QCC - 1))
            nc.scalar.copy(qhT[:, m, qb * 512:(qb + 1) * 512], ps)

    # ---- phase 0b: ctxT + vh interleaved per k row-tile ---------------------
    ctxT = [big.tile([128, NK], F32R, tag="big", name=f"ctxT{j}") for j in range(CCC)]
    khT = proj.tile([128, 2, NK], F32R, tag="khT")  # [d'(2x128), k]
    vha = proj.tile([128, NKT, HPC * HW], F32R, tag="vha")  # [k%128, kt, h*(64+1)]
    ones_f = smal.tile([128, NKT * HPC], F32, tag="ones", bufs=1)
    nc.vector.memset(ones_f, 1.0)
    ones_cols = vha.rearrange("p t (h e) -> p t h e", e=HW)[:, :, :, HEAD_DIM]
    nc.scalar.copy(ones_cols, ones_f.rearrange("p (t h) -> p t h", t=NKT))
    for i in range(NKT):
        cld = ld.tile([128, CTX_DIM], F32R, tag="cld")
        nc.sync.dma_start(out=cld, in_=cx[i * 128:(i + 1) * 128, :].bitcast(F32R))
        for j in range(CCC):
            transpose_into(ctxT[j][:, i * 128:(i + 1) * 128],
                           cld[:, j * 128:(j + 1) * 128], rot)
            rot += 1
        # vh row-tile i only needs ctxT[:, i-slice], which is now in flight
        ps = ps_av.tile([128, DH], F32, tag="av", name=f"vps{i}")
        for cc in range(CCC):
            nc.tensor.matmul(
                ps,
                lhsT=ctxT[cc][:, i * 128:(i + 1) * 128],
                rhs=wv_t[:, cc, :],
                start=(cc == 0), stop=(cc == CCC - 1))
        dst = vha.rearrange("p t (h e) -> p t h e", e=HW)[:, i, :, :HEAD_DIM]
        nc.scalar.copy(dst, ps)
        if i % 4 == 3:
            # kh^T for this 512-wide k block only needs the 4 row-tiles just
            # transposed, so the B-side exp pipeline can start early.
            kb = i // 4
            for m in range(2):
                ps2 = ps_tr.tile([128, 512], F32, tag="tr", name=f"kp{kb}_{m}")
                for cc in range(CCC):
                    nc.tensor.matmul(
                        ps2,
                        lhsT=wk_t[:, cc, m * 128:(m + 1) * 128],
                        rhs=ctxT[cc][:, kb * 512:(kb + 1) * 512],
                        start=(cc == 0), stop=(cc == CCC - 1))
                nc.scalar.copy(khT[:, m, kb * 512:(kb + 1) * 512], ps2)

    ps_tr_cm.__exit__(None, None, None)
    ps_st = ctx.enter_context(tc.tile_pool(name="ps_st", bufs=2, space="PSUM"))
    ps_s = ctx.enter_context(tc.tile_pool(name="ps_s", bufs=1, space="PSUM"))

    # ---- main loop: heads software-pipelined with lag 1 ---------------------
    # B(h) produces softmax denominators (ones column); A(h) consumes them.
    # A(h-1) score tiles are interleaved between B(h) S^T tiles at 1:2 ratio
    # so the in-order ACT queue always has ready exp work.
    rrs_by_head = {}
    av_by_head = {}

    def head_mo(h):
        return (h * HEAD_DIM) % 128, (h * HEAD_DIM) // 128

    def emit_b_kt(h, kt):
        off, m = head_mo(h)
        av_ps = av_by_head[h]
        st_ps = ps_st.tile([128, NQ], F32, tag="st", name=f"st{h}_{kt}")
        for qb in range(NQB):
            nc.tensor.matmul(
                st_ps[:, qb * 512:(qb + 1) * 512],
                lhsT=khT[off:off + HEAD_DIM, m, kt * 128:(kt + 1) * 128],
                rhs=qhT[off:off + HEAD_DIM, m, qb * 512:(qb + 1) * 512],
                start=True, stop=True)
        est = med.tile([128, NQ], F32R, tag="med", name=f"est{h}_{kt}")
        nc.scalar.activation(out=est, in_=st_ps, func=AF.Exp, scale=SCALE)
        for qb in range(NQB):
            nc.tensor.matmul(
                av_ps[qb],
                lhsT=vha[:, kt, h * HW:(h + 1) * HW],
                rhs=est[:, qb * 512:(qb + 1) * 512],
                start=(kt == 0), stop=(kt == NKT - 1))

    def emit_b_tail(h):
        av_ps = av_by_head.pop(h)
        rrs = []
        for qb in range(NQB):
            cv = smal.tile([HW, 512], F32, tag="cv", name=f"cv{h}_{qb}")
            nc.vector.tensor_copy(cv, av_ps[qb])
            rr = smal.tile([1, 512], F32, tag="rr", name=f"rr{h}_{qb}", bufs=4)
            rrs.append(rr)
            nc.vector.reciprocal(rr, cv[HEAD_DIM:HW, :])
            cvf = smal.tile([HEAD_DIM, 512], F32, tag="cvf", name=f"cvf{h}_{qb}")
            rrb = smal.tile([HEAD_DIM, 512], F32, tag="rrb", name=f"rrb{h}_{qb}")
            nc.gpsimd.partition_broadcast(rrb, rr)
            nc.vector.tensor_tensor(out=cvf, in0=cv[:HEAD_DIM, :], in1=rrb,
                                    op=mybir.AluOpType.mult)
            nc.sync.dma_start(
                out=cvt_o[h * HEAD_DIM:(h + 1) * HEAD_DIM,
                          qb * 512:(qb + 1) * 512],
                in_=cvf)
        rrs_by_head[h] = rrs

    def emit_a_qt(h, qt):
        off, m = head_mo(h)
        rrs = rrs_by_head[h]
        rcp = smal.tile([128, 1], F32, tag="rcp", name=f"rcp{h}_{qt}", bufs=4)
        nc.sync.dma_start(
            out=rcp,
            in_=rrs[qt // 4][0:1, (qt % 4) * 128:(qt % 4) * 128 + 128])
        exps = big.tile([128, NK], F32, tag="big", name=f"exps{h}_{qt}")
        for half in range(2):
            s_ps = ps_s.tile([128, NK // 2], F32, tag="s",
                             name=f"s{h}_{qt}_{half}")
            for kb2 in range(2):
                kb = half * 2 + kb2
                nc.tensor.matmul(
                    s_ps[:, kb2 * 512:(kb2 + 1) * 512],
                    lhsT=qhT[off:off + HEAD_DIM, m, qt * 128:(qt + 1) * 128],
                    rhs=khT[off:off + HEAD_DIM, m, kb * 512:(kb + 1) * 512],
                    start=True, stop=True)
            nc.scalar.activation(
                out=exps[:, half * (NK // 2):(half + 1) * (NK // 2)],
                in_=s_ps, func=AF.Exp, scale=SCALE)
        attn = big.tile([128, NK], F32, tag="big", name=f"attn{h}_{qt}")
        nc.vector.tensor_scalar_mul(attn, exps, rcp)
        nc.sync.dma_start(out=attn_o[h, qt * 128:(qt + 1) * 128, :], in_=attn)

    for h in range(HPC):
        av_by_head[h] = [ps_av.tile([HW, 512], F32, tag="av", name=f"av{h}_{qb}")
                         for qb in range(NQB)]
        for kt in range(NKT):
            emit_b_kt(h, kt)
            if h >= 1 and kt % 2 == 1:
                emit_a_qt(h - 1, kt // 2)
        emit_b_tail(h)
    for qt in range(NQT):
        emit_a_qt(HPC - 1, qt)


def kernel(q, context, Wq, Wk, Wv):
    global _CACHED_NC
    q = np.asarray(q, dtype=np.float32)
    context = np.asarray(context, dtype=np.float32)
    Wq = np.asarray(Wq, dtype=np.float32)
    Wk = np.asarray(Wk, dtype=np.float32)
    Wv = np.asarray(Wv, dtype=np.float32)

    if _CACHED_NC is None:
        _CACHED_NC = _build_nc()
    nc = _CACHED_NC

    in_maps = []
    for c in range(N_CORES):
        b, hg = c // 2, c % 2
        cols = slice(hg * DH, (hg + 1) * DH)
        in_maps.append({
            "qx": np.ascontiguousarray(q[b]),
            "cx": np.ascontiguousarray(context[b]),
            "wq": np.ascontiguousarray(Wq[:, cols]),
            "wk": np.ascontiguousarray(Wk[:, cols]),
            "wv": np.ascontiguousarray(Wv[:, cols]),
        })

    res = run_bass_kernel_spmd(nc, in_maps, list(range(N_CORES)))

    attn = np.empty((B, NUM_HEADS, NQ, NK), np.float32)
    ctxv = np.empty((B, NQ, DIM), np.float32)
    for c in range(N_CORES):
        b, hg = c // 2, c % 2
        attn[b, hg * HPC:(hg + 1) * HPC] = res.results[c]["attn_o"]
        ctxv[b, :, hg * DH:(hg + 1) * DH] = res.results[c]["cvt_o"].T
    return ctxv, attn
